# revision 3
# baseline (speedup 1.0000x reference)
"""Trainium2 Bass kernel for nn_CausalWanModel (frame-block-causal attention).

Self-contained: hardcodes shapes from the problem spec.
  B=1, T=3120, D=1536, H=12 heads, hd=128, frame_seqlen=780, 8 cores.

Sharding:
  k/v projections: sequence-parallel, core c owns tokens [390c, 390c+390).
    k/v are exchanged via AllGathers split in thirds (head groups of 4 for
    k, column groups of 512 for v) so the collective pipeline overlaps the
    q projection and early attention heads.
  queries: interleaved frame sharding. Each frame of 780 tokens is split
    across the 8 cores ([780f + 390c/4, 780f + 390(c+1)/4)), so every core
    owns 4 "frame groups" of <=98 query tokens, one per frame. The
    frame-block-causal mask then becomes a static key-prefix length per
    group (frame f attends to keys [0, 780(f+1))) -- no mask tensors, no
    wasted matmul/exp/add work, and the program is identical on all cores.

Keys are tiled 24x128+48 over the gathered 3120 tokens (no padding). The
only partial tiles are the frame-boundary tiles {6,12,18,24} where just
the first {12,24,36,48} key rows participate for the ending frame.

Matmuls run in bf16 (fp32 PSUM accumulation); RMSNorm statistics in fp32.
"""

import math

import numpy as np
import ml_dtypes

import concourse.bacc as bacc
import concourse.mybir as mybir
import concourse.tile as tile
from concourse.bass_utils import run_bass_kernel_spmd

F32 = mybir.dt.float32
BF16 = mybir.dt.bfloat16

NC = 8
T = 3120
D = 1536
H = 12
HD = 128
L = 780  # frame_seqlen
CHUNK = T // NC  # 390 kv tokens per core
KC = D // 128  # 12 contraction chunks
EPS = 1e-6
SCALE = 1.0 / math.sqrt(HD)

QG = 98                 # query slots per frame group (97 or 98 real)
QW = 4 * QG             # 392 query slots per core
NKT = 25                # key tiles over 3120 gathered keys: 24x128 + 48
KSZ = [128] * 24 + [48]
BT = [6, 12, 18, 24]    # frame f's boundary (partial) key tile
NV = [12, 24, 36, 48]   # valid rows of that partial tile
COLG = [(g * 512, 512) for g in range(3)]
TOK_SUBS_KV = [(0, 128), (128, 128), (256, 128), (384, 6)]
TOK_SUBS_Q = [(0, 128), (128, 128), (256, 128), (384, 8)]


def _fmin(kt):
    """Smallest frame group treating key tile kt as full, else None."""
    for f in range(4):
        if kt < BT[f]:
            return f
    return None


def _pfrm(kt):
    """Frame group for which kt is the partial boundary tile, else None."""
    return BT.index(kt) if kt in BT else None


def build_kernel(apply_bias_qk=False, apply_g=False, apply_bias_v=False,
                 apply_bias_o=False, debug=False):
    nc = bacc.Bacc("TRN2", target_bir_lowering=False, debug=False, num_devices=NC)

    # ---- I/O ----
    xT_kv = nc.dram_tensor("xT_kv", [D, CHUNK], BF16, kind="ExternalInput")
    xT_q = nc.dram_tensor("xT_q", [D, QW], BF16, kind="ExternalInput")
    # weights pre-tiled on host: wq/wk[d] = [128, KC*128] (lhsT chunks),
    # wv/wo[g] = [128, KC*512] (rhs chunks per column group)
    wq = nc.dram_tensor("wq", [KC, 128, KC * 128], BF16, kind="ExternalInput")
    wk = nc.dram_tensor("wk", [KC, 128, KC * 128], BF16, kind="ExternalInput")
    wv = nc.dram_tensor("wv", [3, 128, KC * 512], BF16, kind="ExternalInput")
    wo = nc.dram_tensor("wo", [3, 128, KC * 512], BF16, kind="ExternalInput")
    costk = nc.dram_tensor("costk", [128, CHUNK], F32, kind="ExternalInput")
    sintk = nc.dram_tensor("sintk", [128, CHUNK], F32, kind="ExternalInput")
    costq = nc.dram_tensor("costq", [128, QW], F32, kind="ExternalInput")
    sintq = nc.dram_tensor("sintq", [128, QW], F32, kind="ExternalInput")
    bqk2 = nc.dram_tensor("bqk2", [2 * KC, 128], F32, kind="ExternalInput")
    gqk2 = nc.dram_tensor("gqk2", [2 * KC, 128], F32, kind="ExternalInput")
    bvo = nc.dram_tensor("bvo", [2, D], F32, kind="ExternalInput")
    out_part = nc.dram_tensor("out_part", [QW, D], F32, kind="ExternalOutput")

    # ---- collective buffers (k in thirds by head group, v by col group) ----
    k_in = [nc.dram_tensor(f"k_in{i}", [512 * CHUNK], BF16) for i in range(3)]
    v_in = [nc.dram_tensor(f"v_in{i}", [CHUNK * 512], BF16) for i in range(3)]
    k_out = [nc.dram_tensor(f"k_out{i}", [NC, 512 * CHUNK], BF16,
                            addr_space="Shared") for i in range(3)]
    v_out = [nc.dram_tensor(f"v_out{i}", [NC, CHUNK * 512], BF16,
                            addr_space="Shared") for i in range(3)]

    if debug:
        dbg_qT = nc.dram_tensor("dbg_qT", [128, KC * QW], F32, kind="ExternalOutput")
        dbg_kT = nc.dram_tensor("dbg_kT", [128, KC * CHUNK], F32,
                                kind="ExternalOutput")
        dbg_sums = nc.dram_tensor("dbg_sums", [H, QW], F32, kind="ExternalOutput")

    with tile.TileContext(nc) as tc:
        with tc.tile_pool(name="const", bufs=1) as cpool:
            xkv_sb = cpool.tile([128, KC * CHUNK], BF16, tag="xkv_sb")
            xq_sb = cpool.tile([128, KC * QW], BF16, tag="xq_sb")
            qT_sb = cpool.tile([128, H * QW], BF16, tag="qT_sb")
            attnT_sb = cpool.tile([128, H * QW], BF16, tag="attnT_sb")
            costk_sb = cpool.tile([128, CHUNK], F32, tag="costk_sb")
            sintk_sb = cpool.tile([128, CHUNK], F32, tag="sintk_sb")
            costq_sb = cpool.tile([128, QW], F32, tag="costq_sb")
            sintq_sb = cpool.tile([128, QW], F32, tag="sintq_sb")
            ones_f32 = cpool.tile([128, 1], F32, tag="ones_f32")
            ones_bf = cpool.tile([128, 1], BF16, tag="ones_bf")
            sq_scale = cpool.tile([1, QW], F32, tag="sq_scale")
            sk_scale = cpool.tile([1, CHUNK], F32, tag="sk_scale")
            sq_bc = cpool.tile([128, QW], F32, tag="sq_bc")
            sk_bc = cpool.tile([128, CHUNK], F32, tag="sk_bc")

            eps_sb = cpool.tile([1, 1], F32, tag="eps_sb")
            nc.gpsimd.memset(ones_f32[:, :], 1.0)
            nc.gpsimd.memset(ones_bf[:, :], 1.0)
            nc.gpsimd.memset(eps_sb[:, :], EPS)

            for d in range(KC):
                nc.scalar.dma_start(out=xkv_sb[:, d * CHUNK:(d + 1) * CHUNK],
                                    in_=xT_kv[d * 128:(d + 1) * 128, :])
                nc.scalar.dma_start(out=xq_sb[:, d * QW:(d + 1) * QW],
                                    in_=xT_q[d * 128:(d + 1) * 128, :])
            nc.sync.dma_start(out=costk_sb[:, :], in_=costk[:, :])
            nc.sync.dma_start(out=sintk_sb[:, :], in_=sintk[:, :])
            nc.sync.dma_start(out=costq_sb[:, :], in_=costq[:, :])
            nc.sync.dma_start(out=sintq_sb[:, :], in_=sintq[:, :])
            bqk_sb = gqk_sb = bvo_sb = None
            if apply_bias_qk:
                bqk_sb = cpool.tile([128, 2 * KC], F32, tag="bqk_sb")
                nc.sync.dma_start(out=bqk_sb[:, :],
                                  in_=bqk2.ap().rearrange("c p -> p c"))
            if apply_g:
                gqk_sb = cpool.tile([128, 2 * KC], F32, tag="gqk_sb")
                nc.sync.dma_start(out=gqk_sb[:, :],
                                  in_=gqk2.ap().rearrange("c p -> p c"))
            if apply_bias_v or apply_bias_o:
                bvo_sb = cpool.tile([2, D], F32, tag="bvo_sb")
                nc.sync.dma_start(out=bvo_sb[:, :], in_=bvo[:, :])

            # ===== Phase 1: projections + rmsnorm + rope =====
            with tc.tile_pool(name="p1sb", bufs=3) as p1sb, \
                 tc.tile_pool(name="p1w", bufs=5) as p1w, \
                 tc.tile_pool(name="upool", bufs=1) as upool, \
                 tc.tile_pool(name="p1ps", bufs=3, space="PSUM") as p1ps, \
                 tc.tile_pool(name="ssqps", bufs=1, space="PSUM") as ssqps:

                u_tiles = {(name, d): upool.tile(
                    [128, QW if name == "q" else CHUNK], F32,
                    name=f"u_{name}_{d}", tag=f"u_{name}_{d}")
                           for name in ("q", "k") for d in range(KC)}
                ssq_ps = {}

                def qk_proj(name, w, is_q):
                    width = QW if is_q else CHUNK
                    xsrc = xq_sb if is_q else xkv_sb
                    ssq_ps[name] = ssqps.tile([1, width], F32, name=f"ssq_{name}",
                                              tag=f"ssq_{name}")
                    for d in range(KC):
                        wt = p1w.tile([128, D], BF16, tag="wqk_t", name="wqk_t")
                        nc.sync.dma_start(out=wt[:, :], in_=w[d, :, :])
                        ps = p1ps.tile([128, width], F32, tag="proj_ps",
                                       name="proj_ps")
                        for c in range(KC):
                            nc.tensor.matmul(
                                ps[:, :],
                                wt[:, c * 128:(c + 1) * 128],
                                xsrc[:, c * width:(c + 1) * width],
                                start=(c == 0), stop=(c == KC - 1))
                        ur = u_tiles[(name, d)]
                        if apply_bias_qk:
                            bias_col = (0 if is_q else KC) + d
                            nc.vector.tensor_scalar_add(
                                ur[:, :], ps[:, :], bqk_sb[:, bias_col:bias_col + 1])
                        else:
                            nc.scalar.copy(ur[:, :], ps[:, :])
                        sq = p1sb.tile([128, width], BF16, tag="sqsb", name="sqsb")
                        nc.vector.tensor_tensor(sq[:, :], ur[:, :], ur[:, :],
                                                mybir.AluOpType.mult)
                        nc.tensor.matmul(ssq_ps[name][:, :], ones_bf[:, :], sq[:, :],
                                         start=(d == 0), stop=(d == KC - 1))

                def qk_scales(name, stile, sbc):
                    # 1/sqrt(mean(u^2) + eps) in one activation
                    nc.scalar.activation(stile[:, :], ssq_ps[name][:, :],
                                         mybir.ActivationFunctionType.Rsqrt,
                                         bias=eps_sb[:, :], scale=1.0 / D)
                    nc.gpsimd.partition_broadcast(sbc[:, :], stile[:, :])

                def qk_rope(name, sbc, d):
                    is_q = name == "q"
                    width = QW if is_q else CHUNK
                    cost_sb = costq_sb if is_q else costk_sb
                    sint_sb = sintq_sb if is_q else sintk_sb
                    ur = u_tiles[(name, d)]
                    qs = p1sb.tile([128, width], F32, tag="qs", name="qs")
                    nc.vector.tensor_tensor(qs[:, :], ur[:, :], sbc[:, :],
                                            mybir.AluOpType.mult)
                    if apply_g:
                        gcol = (0 if is_q else KC) + d
                        nc.vector.tensor_scalar_mul(
                            qs[:, :], qs[:, :], gqk_sb[:, gcol:gcol + 1])
                    qsw = p1sb.tile([128, width], F32, tag="qsw", name="qsw")
                    nc.scalar.dma_start(out=qsw[0:64, :], in_=qs[64:128, :])
                    nc.scalar.dma_start(out=qsw[64:128, :], in_=qs[0:64, :])
                    t1 = p1sb.tile([128, width], F32, tag="rope_t1", name="rope_t1")
                    t2 = p1sb.tile([128, width], F32, tag="rope_t2", name="rope_t2")
                    nc.vector.tensor_tensor(t1[:, :], qs[:, :], cost_sb[:, :],
                                            mybir.AluOpType.mult)
                    nc.vector.tensor_tensor(t2[:, :], qsw[:, :], sint_sb[:, :],
                                            mybir.AluOpType.mult)
                    if is_q:
                        dst = qT_sb[:, d * QW:(d + 1) * QW]
                        nc.vector.tensor_tensor(dst[:, :], t1[:, :], t2[:, :],
                                                mybir.AluOpType.add)
                        if debug:
                            df = p1sb.tile([128, width], F32, tag="dbgf",
                                           name="dbgf")
                            nc.vector.tensor_copy(df[:, :], dst)
                            nc.sync.dma_start(
                                out=dbg_qT[:, d * QW:(d + 1) * QW], in_=df[:, :])
                    else:
                        kr = p1sb.tile([128, width], BF16, tag="krope",
                                       name="krope")
                        nc.vector.tensor_tensor(kr[:, :], t1[:, :], t2[:, :],
                                                mybir.AluOpType.add)
                        # k_in third i holds k rows [512i, 512i+512) x CHUNK
                        third = d // 4
                        row0 = (d % 4) * 128
                        nc.sync.dma_start(
                            out=k_in[third].ap()
                            .rearrange("(r t) -> r t", t=CHUNK)
                            [row0:row0 + 128, :],
                            in_=kr[:, :])
                        if debug:
                            df = p1sb.tile([128, width], F32, tag="dbgf",
                                           name="dbgf")
                            nc.vector.tensor_copy(df[:, :], kr[:, :])
                            nc.sync.dma_start(
                                out=dbg_kT[:, d * CHUNK:(d + 1) * CHUNK],
                                in_=df[:, :])

                def allgather(src, dst):
                    nc.gpsimd.collective_compute(
                        "AllGather", mybir.AluOpType.bypass,
                        ins=[src.ap().opt()],
                        outs=[dst.ap().opt()],
                        replica_groups=[list(range(NC))],
                    )

                # ---- k first: proj, scales, rope in thirds, AG per third ----
                qk_proj("k", wk, False)
                qk_scales("k", sk_scale, sk_bc)
                for d in range(4):
                    qk_rope("k", sk_bc, d)
                allgather(k_in[0], k_out[0])
                for d in range(4, 8):
                    qk_rope("k", sk_bc, d)

                # ---- v projection per column group, AG after each ----
                for gi, (c0, csz) in enumerate(COLG):
                    wt = p1w.tile([128, KC * 512], BF16, tag="wv_t", name="wv_t")
                    nc.sync.dma_start(out=wt[:, :], in_=wv[gi, :, :])
                    for (t0, tsz) in TOK_SUBS_KV:
                        ps = p1ps.tile([128, 512], F32, tag="v_ps", name="v_ps")
                        for c in range(KC):
                            nc.tensor.matmul(
                                ps[0:tsz, :],
                                xkv_sb[:, c * CHUNK + t0:c * CHUNK + t0 + tsz],
                                wt[:, c * 512:(c + 1) * 512],
                                start=(c == 0), stop=(c == KC - 1))
                        vsb = p1sb.tile([128, 512], BF16, tag="vsb", name="vsb")
                        if apply_bias_v:
                            bvb = p1sb.tile([128, 512], F32, tag="bvb", name="bvb")
                            nc.gpsimd.partition_broadcast(
                                bvb[:, :], bvo_sb[0:1, c0:c0 + csz])
                            nc.vector.tensor_tensor(
                                vsb[0:tsz, :], ps[0:tsz, :], bvb[0:tsz, :],
                                mybir.AluOpType.add)
                        else:
                            nc.vector.tensor_copy(vsb[0:tsz, :], ps[0:tsz, :])
                        nc.sync.dma_start(
                            out=v_in[gi].ap().rearrange("(t c) -> t c", c=512)
                            [t0:t0 + tsz, :],
                            in_=vsb[0:tsz, :])
                    if gi == 0:
                        # finish remaining k rope, then stream AGs:
                        # k0 (already queued), v0, k1, v1, k2, v2
                        for d in range(8, KC):
                            qk_rope("k", sk_bc, d)
                        allgather(v_in[0], v_out[0])
                        allgather(k_in[1], k_out[1])
                    elif gi == 1:
                        allgather(v_in[1], v_out[1])
                        allgather(k_in[2], k_out[2])
                    else:
                        allgather(v_in[2], v_out[2])

                # ---- q last (overlaps the collectives) ----
                qk_proj("q", wq, True)
                qk_scales("q", sq_scale, sq_bc)
                for d in range(KC):
                    qk_rope("q", sq_bc, d)

            # =========== Phase 2: attention ===========
            with tc.tile_pool(name="p4w", bufs=1) as p4w:
              with tc.tile_pool(name="a_k", bufs=2) as akp, \
                   tc.tile_pool(name="a_v", bufs=2) as avp, \
                   tc.tile_pool(name="a_p", bufs=4) as app, \
                   tc.tile_pool(name="a_sb", bufs=3) as asb, \
                   tc.tile_pool(name="a_ps", bufs=2, space="PSUM") as aps, \
                   tc.tile_pool(name="acc_ps", bufs=2, space="PSUM") as accps, \
                   tc.tile_pool(name="sum_ps", bufs=1, space="PSUM") as sumps:
                # prefetch Wo column groups during attention
                wo_tiles = []
                for gi, (c0, csz) in enumerate(COLG):
                    wt = p4w.tile([128, KC * 512], BF16, tag=f"wo_t{gi}",
                                  name=f"wo_t{gi}")
                    nc.sync.dma_start(out=wt[:, :], in_=wo[gi, :, :])
                    wo_tiles.append(wt)

                for h in range(H):
                    # keys for head h: [128 hd, 3120 keys]
                    kthird = h // 4
                    krow = (h % 4) * 128
                    kt_sb = akp.tile([128, T], BF16, tag="kt_sb", name="kt_sb")
                    for r in range(NC):
                        nc.sync.dma_start(
                            out=kt_sb[:, r * CHUNK:(r + 1) * CHUNK],
                            in_=k_out[kthird].ap()[r, :]
                            .rearrange("(row t) -> row t", t=CHUNK)
                            [krow:krow + 128, :])
                    # values for head h: [128 slot-in-tile, 25 tiles, 128 hd]
                    vthird = (h * 128) // 512
                    vcol = (h * 128) % 512
                    vt_sb = avp.tile([128, NKT, 128], BF16, tag="vt_sb",
                                     name="vt_sb")
                    vflat = (v_out[vthird].ap()
                             .rearrange("r (t c) -> (r t) c", c=512)
                             [:, vcol:vcol + 128])
                    nc.sync.dma_start(
                        out=vt_sb[:, 0:24, :],
                        in_=vflat[0:3072, :].rearrange("(t p) c -> p t c", p=128))
                    nc.sync.dma_start(
                        out=vt_sb[0:48, 24, :], in_=vflat[3072:T, :])

                    qh = qT_sb[:, h * QW:(h + 1) * QW]
                    acc = accps.tile([128, QW], F32, tag="acc", name="acc")
                    sumacc = asb.tile([128, QW], F32, tag="sumacc", name="sumacc")
                    nc.vector.memset(sumacc[:, :], 0.0)

                    for b in range(13):  # key-tile batches {0,1},...,{24}
                        kts = [2 * b, 2 * b + 1] if b < 12 else [24]
                        sc = aps.tile([128, 2, 512], F32, tag="sc", name="sc")
                        fmin = _fmin(kts[0])  # same within a batch (or None)
                        for j, kt in enumerate(kts):
                            ksz = KSZ[kt]
                            if fmin is not None:
                                nc.tensor.matmul(
                                    sc[0:ksz, j, QG * fmin:QW],
                                    kt_sb[:, 128 * kt:128 * kt + ksz],
                                    qh[:, QG * fmin:QW],
                                    start=True, stop=True)
                            pf = _pfrm(kt)
                            if pf is not None:
                                nvv = NV[pf]
                                nc.tensor.matmul(
                                    sc[0:nvv, j, QG * pf:QG * pf + QG],
                                    kt_sb[:, 128 * kt:128 * kt + nvv],
                                    qh[:, QG * pf:QG * pf + QG],
                                    start=True, stop=True)
                        pr = app.tile([128, 2, QW], BF16, tag="pr", name="pr")
                        if fmin is not None:
                            nc.scalar.activation(
                                pr[:, 0:len(kts), QG * fmin:QW],
                                sc[:, 0:len(kts), QG * fmin:QW],
                                mybir.ActivationFunctionType.Exp, scale=SCALE)
                        for j, kt in enumerate(kts):
                            pf = _pfrm(kt)
                            if pf is not None:
                                nvv = NV[pf]
                                nc.scalar.activation(
                                    pr[0:nvv, j, QG * pf:QG * pf + QG],
                                    sc[0:nvv, j, QG * pf:QG * pf + QG],
                                    mybir.ActivationFunctionType.Exp, scale=SCALE)
                        for j, kt in enumerate(kts):
                            ksz = KSZ[kt]
                            if fmin is not None:
                                nc.tensor.matmul(
                                    acc[:, QG * fmin:QW],
                                    vt_sb[0:ksz, kt, :],
                                    pr[0:ksz, j, QG * fmin:QW],
                                    start=(kt == 0), stop=False)
                                nc.vector.tensor_tensor(
                                    sumacc[0:ksz, QG * fmin:QW],
                                    sumacc[0:ksz, QG * fmin:QW],
                                    pr[0:ksz, j, QG * fmin:QW],
                                    mybir.AluOpType.add)
                            pf = _pfrm(kt)
                            if pf is not None:
                                nvv = NV[pf]
                                nc.tensor.matmul(
                                    acc[:, QG * pf:QG * pf + QG],
                                    vt_sb[0:nvv, kt, :],
                                    pr[0:nvv, j, QG * pf:QG * pf + QG],
                                    start=False, stop=True)
                                nc.vector.tensor_tensor(
                                    sumacc[0:nvv, QG * pf:QG * pf + QG],
                                    sumacc[0:nvv, QG * pf:QG * pf + QG],
                                    pr[0:nvv, j, QG * pf:QG * pf + QG],
                                    mybir.AluOpType.add)

                    sums = sumps.tile([1, QW], F32, tag="sums", name="sums")
                    nc.tensor.matmul(sums[:, :], ones_f32[:, :], sumacc[:, :],
                                     start=True, stop=True)
                    sumb = asb.tile([128, QW], F32, tag="sumb", name="sumb")
                    nc.gpsimd.partition_broadcast(sumb[:, :], sums[:, :])
                    nc.vector.tensor_tensor(
                        attnT_sb[:, h * QW:(h + 1) * QW],
                        acc[:, :], sumb[:, :],
                        mybir.AluOpType.divide)
                    if debug:
                        ssb = asb.tile([1, QW], F32, tag="ssb", name="ssb")
                        nc.vector.tensor_copy(ssb[:, :], sums[:, :])
                        nc.sync.dma_start(out=dbg_sums[h:h + 1, :], in_=ssb[:, :])

            # =========== Phase 3: o-projection ===========
              with tc.tile_pool(name="p4sb", bufs=3) as p4sb, \
                   tc.tile_pool(name="p4ps", bufs=3, space="PSUM") as p4ps:
                  for gi, (c0, csz) in enumerate(COLG):
                      wt = wo_tiles[gi]
                      for (t0, tsz) in TOK_SUBS_Q:
                          ps = p4ps.tile([128, 512], F32, tag="o_ps", name="o_ps")
                          for hh in range(KC):
                              nc.tensor.matmul(
                                  ps[0:tsz, :],
                                  attnT_sb[:, hh * QW + t0:hh * QW + t0 + tsz],
                                  wt[:, hh * 512:(hh + 1) * 512],
                                  start=(hh == 0), stop=(hh == KC - 1))
                          osb = p4sb.tile([128, 512], F32, tag="osb", name="osb")
                          if apply_bias_o:
                              bob = p4sb.tile([128, 512], F32, tag="bob",
                                              name="bob")
                              nc.gpsimd.partition_broadcast(
                                  bob[:, :], bvo_sb[1:2, c0:c0 + csz])
                              nc.vector.tensor_tensor(
                                  osb[0:tsz, :], ps[0:tsz, :], bob[0:tsz, :],
                                  mybir.AluOpType.add)
                          else:
                              nc.vector.tensor_copy(osb[0:tsz, :], ps[0:tsz, :])
                          nc.sync.dma_start(
                              out=out_part[t0:t0 + tsz, c0:c0 + csz],
                              in_=osb[0:tsz, :])

    nc.compile()
    return nc


_NC_CACHE = {}


def _get_nc(key):
    if key not in _NC_CACHE:
        _NC_CACHE[key] = build_kernel(*key)
    return _NC_CACHE[key]


def _q_slices(c):
    """Per-core query token indices: 4 frame groups of QG slots (junk-padded)."""
    lo, hi = (CHUNK * c) // 4, (CHUNK * (c + 1)) // 4  # within-frame range
    idx = []
    for f in range(4):
        toks = list(range(L * f + lo, L * f + hi))
        toks += [toks[-1]] * (QG - len(toks))  # pad junk slots
        idx.extend(toks)
    return np.array(idx), hi - lo


def _prep_inputs(x, freqs_cos, freqs_sin, Wq, bq, Wk, bk, Wv, bv, Wo, bo,
                 gq, gk, frame_seqlen, debug=False):
    assert int(frame_seqlen) == L
    x2d = np.asarray(x, np.float32).reshape(T, D)
    xT_full = np.ascontiguousarray(x2d.T)

    perm = np.concatenate([
        np.concatenate([np.arange(0, 128, 2), np.arange(1, 128, 2)]) + 128 * h
        for h in range(H)])
    Wqp = np.asarray(Wq, np.float32)[:, perm]
    Wkp = np.asarray(Wk, np.float32)[:, perm]
    bqp = np.asarray(bq, np.float32)[perm]
    bkp = np.asarray(bk, np.float32)[perm]
    gqp = np.asarray(gq, np.float32)[perm]
    gkp = np.asarray(gk, np.float32)[perm]

    cosT = np.asarray(freqs_cos, np.float32).T
    sinT = np.asarray(freqs_sin, np.float32).T
    costab = np.concatenate([cosT, cosT], 0)   # [128, T]
    sintab = np.concatenate([-sinT, sinT], 0)

    bf16 = ml_dtypes.bfloat16

    apply_bias_qk = not (np.all(bqp == 0) and np.all(bkp == 0))
    apply_g = not (np.all(gqp == 1) and np.all(gkp == 1))
    apply_bias_v = not np.all(np.asarray(bv) == 0)
    apply_bias_o = not np.all(np.asarray(bo) == 0)
    key = (apply_bias_qk, apply_g, apply_bias_v, apply_bias_o, debug)

    def tile_lhsT(w):  # [D, D] -> [KC, 128, KC*128]: out[d, p, c*128+m] = w[c*128+p, d*128+m]
        return np.ascontiguousarray(
            w.reshape(KC, 128, KC, 128).transpose(2, 1, 0, 3)
            .reshape(KC, 128, KC * 128))

    def tile_rhs(w):  # [D, D] -> [3, 128, KC*512]: out[g, p, c*512+m] = w[c*128+p, g*512+m]
        return np.ascontiguousarray(
            w.reshape(KC, 128, 3, 512).transpose(2, 1, 0, 3)
            .reshape(3, 128, KC * 512))

    shared = {
        "wq": tile_lhsT(Wqp).astype(bf16), "wk": tile_lhsT(Wkp).astype(bf16),
        "wv": tile_rhs(np.asarray(Wv, np.float32)).astype(bf16),
        "wo": tile_rhs(np.asarray(Wo, np.float32)).astype(bf16),
        "bqk2": np.concatenate([bqp, bkp]).reshape(2 * KC, 128),
        "gqk2": np.concatenate([gqp, gkp]).reshape(2 * KC, 128),
        "bvo": np.stack([np.asarray(bv, np.float32),
                         np.asarray(bo, np.float32)]),
    }
    in_maps = []
    for c in range(NC):
        t0 = c * CHUNK
        qidx, _ = _q_slices(c)
        in_maps.append({
            **shared,
            "xT_kv": np.ascontiguousarray(xT_full[:, t0:t0 + CHUNK]).astype(bf16),
            "xT_q": np.ascontiguousarray(xT_full[:, qidx]).astype(bf16),
            "costk": np.ascontiguousarray(costab[:, t0:t0 + CHUNK]),
            "sintk": np.ascontiguousarray(sintab[:, t0:t0 + CHUNK]),
            "costq": np.ascontiguousarray(costab[:, qidx]),
            "sintq": np.ascontiguousarray(sintab[:, qidx]),
        })
    return key, in_maps


def _assemble(results):
    out = np.empty((1, T, D), np.float32)
    for c in range(NC):
        qidx, cnt = _q_slices(c)
        part = results[c]["out_part"]
        for f in range(4):
            toks = qidx[f * QG:f * QG + cnt]
            out[0, toks, :] = part[f * QG:f * QG + cnt]
    return out


def kernel(x, freqs_cos, freqs_sin, Wq, bq, Wk, bk, Wv, bv, Wo, bo,
           gq, gk, frame_seqlen):
    key, in_maps = _prep_inputs(x, freqs_cos, freqs_sin, Wq, bq, Wk, bk,
                                Wv, bv, Wo, bo, gq, gk, frame_seqlen)
    nc = _get_nc(key)
    res = run_bass_kernel_spmd(nc, in_maps, core_ids=list(range(NC)))
    return _assemble(res.results)


# revision 9
# speedup vs baseline: 1.4239x; 1.4239x over previous
"""Trainium2 Bass kernel for nn_CausalWanModel (frame-block-causal attention).

Self-contained: hardcodes shapes from the problem spec.
  B=1, T=3120, D=1536, H=12 heads, hd=128, frame_seqlen=780, 8 cores.

Sharding:
  k/v projections: sequence-parallel, core c owns tokens [390c, 390c+390).
    k/v are exchanged via AllGathers split in thirds (head groups of 4 for
    k, column groups of 512 for v) so the collective pipeline overlaps the
    q projection and early attention heads.
  queries: interleaved frame sharding. Each frame of 780 tokens is split
    across the 8 cores ([780f + 390c/4, 780f + 390(c+1)/4)), so every core
    owns 4 "frame groups" of <=98 query tokens, one per frame. The
    frame-block-causal mask then becomes a static key-prefix length per
    group (frame f attends to keys [0, 780(f+1))) -- no mask tensors, no
    wasted matmul/exp/add work, and the program is identical on all cores.

Keys are tiled 24x128+48 over the gathered 3120 tokens (no padding). The
only partial tiles are the frame-boundary tiles {6,12,18,24} where just
the first {12,24,36,48} key rows participate for the ending frame.

Matmuls run in bf16 (fp32 PSUM accumulation); RMSNorm statistics in fp32.
"""

import math

import numpy as np
import ml_dtypes

import concourse.bacc as bacc
import concourse.mybir as mybir
import concourse.tile as tile
from concourse.bass_utils import run_bass_kernel_spmd

F32 = mybir.dt.float32
BF16 = mybir.dt.bfloat16

NC = 8
T = 3120
D = 1536
H = 12
HD = 128
L = 780  # frame_seqlen
CHUNK = T // NC  # 390 kv tokens per core
KC = D // 128  # 12 contraction chunks
EPS = 1e-6
SCALE = 1.0 / math.sqrt(HD)

QG = 98                 # query slots per frame group (97 or 98 real)
QW = 4 * QG             # 392 query slots per core
NKT = 25                # key tiles over 3120 gathered keys: 24x128 + 48
KSZ = [128] * 24 + [48]
BT = [6, 12, 18, 24]    # frame f's boundary (partial) key tile
NV = [12, 24, 36, 48]   # valid rows of that partial tile
COLG = [(g * 512, 512) for g in range(3)]
TOK_SUBS_KV = [(0, 128), (128, 128), (256, 128), (384, 6)]
TOK_SUBS_Q = [(0, 128), (128, 128), (256, 128), (384, 8)]


def _fmin(kt):
    """Smallest frame group treating key tile kt as full, else None."""
    for f in range(4):
        if kt < BT[f]:
            return f
    return None


def _pfrm(kt):
    """Frame group for which kt is the partial boundary tile, else None."""
    return BT.index(kt) if kt in BT else None


def build_kernel(apply_bias_qk=False, apply_g=False, apply_bias_v=False,
                 apply_bias_o=False, debug=False):
    nc = bacc.Bacc("TRN2", target_bir_lowering=False, debug=False, num_devices=NC)

    # ---- I/O ----
    xT_kv = nc.dram_tensor("xT_kv", [D, CHUNK], BF16, kind="ExternalInput")
    xT_q = nc.dram_tensor("xT_q", [D, QW], BF16, kind="ExternalInput")
    # weights pre-tiled on host: wq/wk[d] = [128, KC*128] (lhsT chunks),
    # wv/wo[g] = [128, KC*512] (rhs chunks per column group)
    wq = nc.dram_tensor("wq", [KC, 128, KC * 128], BF16, kind="ExternalInput")
    wk = nc.dram_tensor("wk", [KC, 128, KC * 128], BF16, kind="ExternalInput")
    wv = nc.dram_tensor("wv", [3, 128, KC * 512], BF16, kind="ExternalInput")
    wo = nc.dram_tensor("wo", [3, 128, KC * 512], BF16, kind="ExternalInput")
    costk = nc.dram_tensor("costk", [128, CHUNK], F32, kind="ExternalInput")
    sintk = nc.dram_tensor("sintk", [128, CHUNK], F32, kind="ExternalInput")
    costq = nc.dram_tensor("costq", [128, QW], F32, kind="ExternalInput")
    sintq = nc.dram_tensor("sintq", [128, QW], F32, kind="ExternalInput")
    bqk2 = nc.dram_tensor("bqk2", [2 * KC, 128], F32, kind="ExternalInput")
    gqk2 = nc.dram_tensor("gqk2", [2 * KC, 128], F32, kind="ExternalInput")
    bvo = nc.dram_tensor("bvo", [2, D], F32, kind="ExternalInput")
    out_part = nc.dram_tensor("out_part", [QW, D], F32, kind="ExternalOutput")

    # ---- collective buffers (k in thirds by head group, v by col group) ----
    k_in = [nc.dram_tensor(f"k_in{i}", [512 * CHUNK], BF16) for i in range(3)]
    v_in = [nc.dram_tensor(f"v_in{i}", [CHUNK * 512], BF16) for i in range(3)]
    k_out = [nc.dram_tensor(f"k_out{i}", [NC, 512 * CHUNK], BF16,
                            addr_space="Shared") for i in range(3)]
    v_out = [nc.dram_tensor(f"v_out{i}", [NC, CHUNK * 512], BF16,
                            addr_space="Shared") for i in range(3)]

    if debug:
        dbg_qT = nc.dram_tensor("dbg_qT", [128, KC * QW], F32, kind="ExternalOutput")
        dbg_kT = nc.dram_tensor("dbg_kT", [128, KC * CHUNK], F32,
                                kind="ExternalOutput")
        dbg_sums = nc.dram_tensor("dbg_sums", [H, QW], F32, kind="ExternalOutput")

    with tile.TileContext(nc) as tc:
        with tc.tile_pool(name="const", bufs=1) as cpool:
            xkv_sb = cpool.tile([128, KC * CHUNK], BF16, tag="xkv_sb")
            xq_sb = cpool.tile([128, KC * QW], BF16, tag="xq_sb")
            qT_sb = cpool.tile([128, H * QW], BF16, tag="qT_sb")
            attnT_sb = cpool.tile([128, H * QW], BF16, tag="attnT_sb")
            costk_sb = cpool.tile([128, CHUNK], F32, tag="costk_sb")
            sintk_sb = cpool.tile([128, CHUNK], F32, tag="sintk_sb")
            costq_sb = cpool.tile([128, QW], F32, tag="costq_sb")
            sintq_sb = cpool.tile([128, QW], F32, tag="sintq_sb")
            ones_f32 = cpool.tile([128, 1], F32, tag="ones_f32")
            ones_bf = cpool.tile([128, 1], BF16, tag="ones_bf")
            sq_scale = cpool.tile([1, QW], F32, tag="sq_scale")
            sk_scale = cpool.tile([1, CHUNK], F32, tag="sk_scale")
            sq_bc = cpool.tile([128, QW], F32, tag="sq_bc")
            sk_bc = cpool.tile([128, CHUNK], F32, tag="sk_bc")

            eps_sb = cpool.tile([1, 1], F32, tag="eps_sb")
            nc.gpsimd.memset(ones_f32[:, :], 1.0)
            nc.gpsimd.memset(ones_bf[:, :], 1.0)
            nc.gpsimd.memset(eps_sb[:, :], EPS)

            for d in range(KC):
                nc.scalar.dma_start(out=xkv_sb[:, d * CHUNK:(d + 1) * CHUNK],
                                    in_=xT_kv[d * 128:(d + 1) * 128, :])
                nc.scalar.dma_start(out=xq_sb[:, d * QW:(d + 1) * QW],
                                    in_=xT_q[d * 128:(d + 1) * 128, :])
            nc.sync.dma_start(out=costk_sb[:, :], in_=costk[:, :])
            nc.sync.dma_start(out=sintk_sb[:, :], in_=sintk[:, :])
            nc.sync.dma_start(out=costq_sb[:, :], in_=costq[:, :])
            nc.sync.dma_start(out=sintq_sb[:, :], in_=sintq[:, :])
            bqk_sb = gqk_sb = bvo_sb = None
            if apply_bias_qk:
                bqk_sb = cpool.tile([128, 2 * KC], F32, tag="bqk_sb")
                nc.sync.dma_start(out=bqk_sb[:, :],
                                  in_=bqk2.ap().rearrange("c p -> p c"))
            if apply_g:
                gqk_sb = cpool.tile([128, 2 * KC], F32, tag="gqk_sb")
                nc.sync.dma_start(out=gqk_sb[:, :],
                                  in_=gqk2.ap().rearrange("c p -> p c"))
            if apply_bias_v or apply_bias_o:
                bvo_sb = cpool.tile([2, D], F32, tag="bvo_sb")
                nc.sync.dma_start(out=bvo_sb[:, :], in_=bvo[:, :])

            # ===== Phase 1: projections + rmsnorm + rope =====
            with tc.tile_pool(name="p1sb", bufs=3) as p1sb, \
                 tc.tile_pool(name="p1w", bufs=5) as p1w, \
                 tc.tile_pool(name="upool", bufs=1) as upool, \
                 tc.tile_pool(name="p1ps", bufs=3, space="PSUM") as p1ps, \
                 tc.tile_pool(name="ssqps", bufs=1, space="PSUM") as ssqps:

                u_tiles = {(name, d): upool.tile(
                    [128, QW if name == "q" else CHUNK], F32,
                    name=f"u_{name}_{d}", tag=f"u_{name}_{d}")
                           for name in ("q", "k") for d in range(KC)}
                ssq_ps = {}

                def qk_proj(name, w, is_q):
                    width = QW if is_q else CHUNK
                    xsrc = xq_sb if is_q else xkv_sb
                    ssq_ps[name] = ssqps.tile([1, width], F32, name=f"ssq_{name}",
                                              tag=f"ssq_{name}")
                    for d in range(KC):
                        wt = p1w.tile([128, D], BF16, tag="wqk_t", name="wqk_t")
                        nc.sync.dma_start(out=wt[:, :], in_=w[d, :, :])
                        ps = p1ps.tile([128, width], F32, tag="proj_ps",
                                       name="proj_ps")
                        for c in range(KC):
                            nc.tensor.matmul(
                                ps[:, :],
                                wt[:, c * 128:(c + 1) * 128],
                                xsrc[:, c * width:(c + 1) * width],
                                start=(c == 0), stop=(c == KC - 1))
                        ur = u_tiles[(name, d)]
                        if apply_bias_qk:
                            bias_col = (0 if is_q else KC) + d
                            nc.vector.tensor_scalar_add(
                                ur[:, :], ps[:, :], bqk_sb[:, bias_col:bias_col + 1])
                        else:
                            nc.scalar.copy(ur[:, :], ps[:, :])
                        sq = p1sb.tile([128, width], BF16, tag="sqsb", name="sqsb")
                        nc.vector.tensor_tensor(sq[:, :], ur[:, :], ur[:, :],
                                                mybir.AluOpType.mult)
                        nc.tensor.matmul(ssq_ps[name][:, :], ones_bf[:, :], sq[:, :],
                                         start=(d == 0), stop=(d == KC - 1))

                def qk_scales(name, stile, sbc):
                    nc.scalar.activation(stile[:, :], ssq_ps[name][:, :],
                                         mybir.ActivationFunctionType.Sqrt,
                                         bias=eps_sb[:, :], scale=1.0 / D)
                    nc.vector.reciprocal_approx_fast(stile[:, :], stile[:, :])
                    nc.gpsimd.partition_broadcast(sbc[:, :], stile[:, :])

                def qk_rope(name, sbc, d):
                    is_q = name == "q"
                    width = QW if is_q else CHUNK
                    cost_sb = costq_sb if is_q else costk_sb
                    sint_sb = sintq_sb if is_q else sintk_sb
                    ur = u_tiles[(name, d)]
                    qs = p1sb.tile([128, width], F32, tag="qs", name="qs")
                    nc.vector.tensor_tensor(qs[:, :], ur[:, :], sbc[:, :],
                                            mybir.AluOpType.mult)
                    if apply_g:
                        gcol = (0 if is_q else KC) + d
                        nc.vector.tensor_scalar_mul(
                            qs[:, :], qs[:, :], gqk_sb[:, gcol:gcol + 1])
                    qsw = p1sb.tile([128, width], F32, tag="qsw", name="qsw")
                    nc.scalar.dma_start(out=qsw[0:64, :], in_=qs[64:128, :])
                    nc.scalar.dma_start(out=qsw[64:128, :], in_=qs[0:64, :])
                    t1 = p1sb.tile([128, width], F32, tag="rope_t1", name="rope_t1")
                    t2 = p1sb.tile([128, width], F32, tag="rope_t2", name="rope_t2")
                    nc.vector.tensor_tensor(t1[:, :], qs[:, :], cost_sb[:, :],
                                            mybir.AluOpType.mult)
                    nc.vector.tensor_tensor(t2[:, :], qsw[:, :], sint_sb[:, :],
                                            mybir.AluOpType.mult)
                    if is_q:
                        dst = qT_sb[:, d * QW:(d + 1) * QW]
                        nc.vector.tensor_tensor(dst[:, :], t1[:, :], t2[:, :],
                                                mybir.AluOpType.add)
                        if debug:
                            df = p1sb.tile([128, width], F32, tag="dbgf",
                                           name="dbgf")
                            nc.vector.tensor_copy(df[:, :], dst)
                            nc.sync.dma_start(
                                out=dbg_qT[:, d * QW:(d + 1) * QW], in_=df[:, :])
                    else:
                        kr = p1sb.tile([128, width], BF16, tag="krope",
                                       name="krope")
                        nc.vector.tensor_tensor(kr[:, :], t1[:, :], t2[:, :],
                                                mybir.AluOpType.add)
                        # k_in third i holds k rows [512i, 512i+512) x CHUNK
                        third = d // 4
                        row0 = (d % 4) * 128
                        nc.sync.dma_start(
                            out=k_in[third].ap()
                            .rearrange("(r t) -> r t", t=CHUNK)
                            [row0:row0 + 128, :],
                            in_=kr[:, :])
                        if debug:
                            df = p1sb.tile([128, width], F32, tag="dbgf",
                                           name="dbgf")
                            nc.vector.tensor_copy(df[:, :], kr[:, :])
                            nc.sync.dma_start(
                                out=dbg_kT[:, d * CHUNK:(d + 1) * CHUNK],
                                in_=df[:, :])

                def allgather(src, dst):
                    nc.gpsimd.collective_compute(
                        "AllGather", mybir.AluOpType.bypass,
                        ins=[src.ap().opt()],
                        outs=[dst.ap().opt()],
                        replica_groups=[list(range(NC))],
                    )

                # ---- k first: proj, scales, rope in thirds, AG per third ----
                qk_proj("k", wk, False)
                qk_scales("k", sk_scale, sk_bc)
                for d in range(4):
                    qk_rope("k", sk_bc, d)
                allgather(k_in[0], k_out[0])
                for d in range(4, 8):
                    qk_rope("k", sk_bc, d)

                # ---- v projection per column group, AG after each ----
                for gi, (c0, csz) in enumerate(COLG):
                    wt = p1w.tile([128, KC * 512], BF16, tag="wv_t", name="wv_t")
                    nc.sync.dma_start(out=wt[:, :], in_=wv[gi, :, :])
                    for (t0, tsz) in TOK_SUBS_KV:
                        ps = p1ps.tile([128, 512], F32, tag="v_ps", name="v_ps")
                        for c in range(KC):
                            nc.tensor.matmul(
                                ps[0:tsz, :],
                                xkv_sb[:, c * CHUNK + t0:c * CHUNK + t0 + tsz],
                                wt[:, c * 512:(c + 1) * 512],
                                start=(c == 0), stop=(c == KC - 1))
                        vsb = p1sb.tile([128, 512], BF16, tag="vsb", name="vsb")
                        if apply_bias_v:
                            bvb = p1sb.tile([128, 512], F32, tag="bvb", name="bvb")
                            nc.gpsimd.partition_broadcast(
                                bvb[:, :], bvo_sb[0:1, c0:c0 + csz])
                            nc.vector.tensor_tensor(
                                vsb[0:tsz, :], ps[0:tsz, :], bvb[0:tsz, :],
                                mybir.AluOpType.add)
                        else:
                            nc.vector.tensor_copy(vsb[0:tsz, :], ps[0:tsz, :])
                        nc.sync.dma_start(
                            out=v_in[gi].ap().rearrange("(t c) -> t c", c=512)
                            [t0:t0 + tsz, :],
                            in_=vsb[0:tsz, :])
                    if gi == 0:
                        # finish remaining k rope, then stream AGs:
                        # k0 (already queued), v0, k1, v1, k2, v2
                        for d in range(8, KC):
                            qk_rope("k", sk_bc, d)
                        allgather(v_in[0], v_out[0])
                        allgather(k_in[1], k_out[1])
                    elif gi == 1:
                        allgather(v_in[1], v_out[1])
                        allgather(k_in[2], k_out[2])
                    else:
                        allgather(v_in[2], v_out[2])

                # ---- q last (overlaps the collectives) ----
                qk_proj("q", wq, True)
                qk_scales("q", sq_scale, sq_bc)
                for d in range(KC):
                    qk_rope("q", sq_bc, d)

            # =========== Phase 2: attention ===========
            with tc.tile_pool(name="p4w", bufs=1) as p4w:
              with tc.tile_pool(name="a_k", bufs=2) as akp, \
                   tc.tile_pool(name="a_v", bufs=2) as avp, \
                   tc.tile_pool(name="a_p", bufs=4) as app, \
                   tc.tile_pool(name="a_sb", bufs=3) as asb, \
                   tc.tile_pool(name="a_ps", bufs=2, space="PSUM") as aps, \
                   tc.tile_pool(name="acc_ps", bufs=2, space="PSUM") as accps, \
                   tc.tile_pool(name="sum_ps", bufs=1, space="PSUM") as sumps:
                # prefetch Wo column groups during attention
                wo_tiles = []
                for gi, (c0, csz) in enumerate(COLG):
                    wt = p4w.tile([128, KC * 512], BF16, tag=f"wo_t{gi}",
                                  name=f"wo_t{gi}")
                    nc.sync.dma_start(out=wt[:, :], in_=wo[gi, :, :])
                    wo_tiles.append(wt)

                for h in range(H):
                    # keys for head h: [128 hd, 3120 keys]
                    kthird = h // 4
                    krow = (h % 4) * 128
                    kt_sb = akp.tile([128, T], BF16, tag="kt_sb", name="kt_sb")
                    for r in range(NC):
                        nc.sync.dma_start(
                            out=kt_sb[:, r * CHUNK:(r + 1) * CHUNK],
                            in_=k_out[kthird].ap()[r, :]
                            .rearrange("(row t) -> row t", t=CHUNK)
                            [krow:krow + 128, :])
                    # values for head h: [128 slot-in-tile, 25 tiles, 128 hd]
                    vthird = (h * 128) // 512
                    vcol = (h * 128) % 512
                    vt_sb = avp.tile([128, NKT, 128], BF16, tag="vt_sb",
                                     name="vt_sb")
                    vflat = (v_out[vthird].ap()
                             .rearrange("r (t c) -> (r t) c", c=512)
                             [:, vcol:vcol + 128])
                    nc.sync.dma_start(
                        out=vt_sb[:, 0:24, :],
                        in_=vflat[0:3072, :].rearrange("(t p) c -> p t c", p=128))
                    nc.sync.dma_start(
                        out=vt_sb[0:48, 24, :], in_=vflat[3072:T, :])

                    qh = qT_sb[:, h * QW:(h + 1) * QW]
                    acc = accps.tile([128, QW], F32, tag="acc", name="acc")
                    sumacc = asb.tile([128, QW], F32, tag="sumacc", name="sumacc")
                    nc.vector.memset(sumacc[:, :], 0.0)

                    for b in range(13):  # key-tile batches {0,1},...,{24}
                        kts = [2 * b, 2 * b + 1] if b < 12 else [24]
                        sc = aps.tile([128, 2, 512], F32, tag="sc", name="sc")
                        fmin = _fmin(kts[0])  # same within a batch (or None)
                        for j, kt in enumerate(kts):
                            ksz = KSZ[kt]
                            if fmin is not None:
                                nc.tensor.matmul(
                                    sc[0:ksz, j, QG * fmin:QW],
                                    kt_sb[:, 128 * kt:128 * kt + ksz],
                                    qh[:, QG * fmin:QW],
                                    start=True, stop=True)
                            pf = _pfrm(kt)
                            if pf is not None:
                                nvv = NV[pf]
                                nc.tensor.matmul(
                                    sc[0:nvv, j, QG * pf:QG * pf + QG],
                                    kt_sb[:, 128 * kt:128 * kt + nvv],
                                    qh[:, QG * pf:QG * pf + QG],
                                    start=True, stop=True)
                        pr = app.tile([128, 2, QW], BF16, tag="pr", name="pr")
                        if fmin is not None:
                            nc.scalar.activation(
                                pr[:, 0:len(kts), QG * fmin:QW],
                                sc[:, 0:len(kts), QG * fmin:QW],
                                mybir.ActivationFunctionType.Exp, scale=SCALE)
                        for j, kt in enumerate(kts):
                            pf = _pfrm(kt)
                            if pf is not None:
                                nvv = NV[pf]
                                nc.scalar.activation(
                                    pr[0:nvv, j, QG * pf:QG * pf + QG],
                                    sc[0:nvv, j, QG * pf:QG * pf + QG],
                                    mybir.ActivationFunctionType.Exp, scale=SCALE)
                        for j, kt in enumerate(kts):
                            ksz = KSZ[kt]
                            if fmin is not None:
                                nc.tensor.matmul(
                                    acc[:, QG * fmin:QW],
                                    vt_sb[0:ksz, kt, :],
                                    pr[0:ksz, j, QG * fmin:QW],
                                    start=(kt == 0), stop=False)
                                nc.vector.tensor_tensor(
                                    sumacc[0:ksz, QG * fmin:QW],
                                    sumacc[0:ksz, QG * fmin:QW],
                                    pr[0:ksz, j, QG * fmin:QW],
                                    mybir.AluOpType.add)
                            pf = _pfrm(kt)
                            if pf is not None:
                                nvv = NV[pf]
                                nc.tensor.matmul(
                                    acc[:, QG * pf:QG * pf + QG],
                                    vt_sb[0:nvv, kt, :],
                                    pr[0:nvv, j, QG * pf:QG * pf + QG],
                                    start=False, stop=True)
                                nc.vector.tensor_tensor(
                                    sumacc[0:nvv, QG * pf:QG * pf + QG],
                                    sumacc[0:nvv, QG * pf:QG * pf + QG],
                                    pr[0:nvv, j, QG * pf:QG * pf + QG],
                                    mybir.AluOpType.add)

                    sums = sumps.tile([1, QW], F32, tag="sums", name="sums")
                    nc.tensor.matmul(sums[:, :], ones_f32[:, :], sumacc[:, :],
                                     start=True, stop=True)
                    rec = asb.tile([1, QW], F32, tag="rec", name="rec")
                    nc.scalar.copy(rec[:, :], sums[:, :])
                    nc.vector.reciprocal_approx_fast(rec[:, :], rec[:, :])
                    recb = asb.tile([128, QW], F32, tag="recb", name="recb")
                    nc.gpsimd.partition_broadcast(recb[:, :], rec[:, :])
                    nc.vector.tensor_tensor(
                        attnT_sb[:, h * QW:(h + 1) * QW],
                        acc[:, :], recb[:, :],
                        mybir.AluOpType.mult)
                    if debug:
                        ssb = asb.tile([1, QW], F32, tag="ssb", name="ssb")
                        nc.vector.tensor_copy(ssb[:, :], sums[:, :])
                        nc.sync.dma_start(out=dbg_sums[h:h + 1, :], in_=ssb[:, :])

            # =========== Phase 3: o-projection ===========
              with tc.tile_pool(name="p4sb", bufs=3) as p4sb, \
                   tc.tile_pool(name="p4ps", bufs=3, space="PSUM") as p4ps:
                  for gi, (c0, csz) in enumerate(COLG):
                      wt = wo_tiles[gi]
                      for (t0, tsz) in TOK_SUBS_Q:
                          ps = p4ps.tile([128, 512], F32, tag="o_ps", name="o_ps")
                          for hh in range(KC):
                              nc.tensor.matmul(
                                  ps[0:tsz, :],
                                  attnT_sb[:, hh * QW + t0:hh * QW + t0 + tsz],
                                  wt[:, hh * 512:(hh + 1) * 512],
                                  start=(hh == 0), stop=(hh == KC - 1))
                          osb = p4sb.tile([128, 512], F32, tag="osb", name="osb")
                          if apply_bias_o:
                              bob = p4sb.tile([128, 512], F32, tag="bob",
                                              name="bob")
                              nc.gpsimd.partition_broadcast(
                                  bob[:, :], bvo_sb[1:2, c0:c0 + csz])
                              nc.vector.tensor_tensor(
                                  osb[0:tsz, :], ps[0:tsz, :], bob[0:tsz, :],
                                  mybir.AluOpType.add)
                          else:
                              nc.vector.tensor_copy(osb[0:tsz, :], ps[0:tsz, :])
                          nc.sync.dma_start(
                              out=out_part[t0:t0 + tsz, c0:c0 + csz],
                              in_=osb[0:tsz, :])

    nc.compile()
    return nc


_NC_CACHE = {}


def _get_nc(key):
    if key not in _NC_CACHE:
        _NC_CACHE[key] = build_kernel(*key)
    return _NC_CACHE[key]


def _q_slices(c):
    """Per-core query token indices: 4 frame groups of QG slots (junk-padded)."""
    lo, hi = (CHUNK * c) // 4, (CHUNK * (c + 1)) // 4  # within-frame range
    idx = []
    for f in range(4):
        toks = list(range(L * f + lo, L * f + hi))
        toks += [toks[-1]] * (QG - len(toks))  # pad junk slots
        idx.extend(toks)
    return np.array(idx), hi - lo


def _prep_inputs(x, freqs_cos, freqs_sin, Wq, bq, Wk, bk, Wv, bv, Wo, bo,
                 gq, gk, frame_seqlen, debug=False):
    assert int(frame_seqlen) == L
    x2d = np.asarray(x, np.float32).reshape(T, D)
    xT_full = np.ascontiguousarray(x2d.T)

    perm = np.concatenate([
        np.concatenate([np.arange(0, 128, 2), np.arange(1, 128, 2)]) + 128 * h
        for h in range(H)])
    Wqp = np.asarray(Wq, np.float32)[:, perm]
    Wkp = np.asarray(Wk, np.float32)[:, perm]
    bqp = np.asarray(bq, np.float32)[perm]
    bkp = np.asarray(bk, np.float32)[perm]
    gqp = np.asarray(gq, np.float32)[perm]
    gkp = np.asarray(gk, np.float32)[perm]

    cosT = np.asarray(freqs_cos, np.float32).T
    sinT = np.asarray(freqs_sin, np.float32).T
    costab = np.concatenate([cosT, cosT], 0)   # [128, T]
    sintab = np.concatenate([-sinT, sinT], 0)

    bf16 = ml_dtypes.bfloat16

    apply_bias_qk = not (np.all(bqp == 0) and np.all(bkp == 0))
    apply_g = not (np.all(gqp == 1) and np.all(gkp == 1))
    apply_bias_v = not np.all(np.asarray(bv) == 0)
    apply_bias_o = not np.all(np.asarray(bo) == 0)
    key = (apply_bias_qk, apply_g, apply_bias_v, apply_bias_o, debug)

    def tile_lhsT(w):  # [D, D] -> [KC, 128, KC*128]: out[d, p, c*128+m] = w[c*128+p, d*128+m]
        return np.ascontiguousarray(
            w.reshape(KC, 128, KC, 128).transpose(2, 1, 0, 3)
            .reshape(KC, 128, KC * 128))

    def tile_rhs(w):  # [D, D] -> [3, 128, KC*512]: out[g, p, c*512+m] = w[c*128+p, g*512+m]
        return np.ascontiguousarray(
            w.reshape(KC, 128, 3, 512).transpose(2, 1, 0, 3)
            .reshape(3, 128, KC * 512))

    shared = {
        "wq": tile_lhsT(Wqp).astype(bf16), "wk": tile_lhsT(Wkp).astype(bf16),
        "wv": tile_rhs(np.asarray(Wv, np.float32)).astype(bf16),
        "wo": tile_rhs(np.asarray(Wo, np.float32)).astype(bf16),
        "bqk2": np.concatenate([bqp, bkp]).reshape(2 * KC, 128),
        "gqk2": np.concatenate([gqp, gkp]).reshape(2 * KC, 128),
        "bvo": np.stack([np.asarray(bv, np.float32),
                         np.asarray(bo, np.float32)]),
    }
    in_maps = []
    for c in range(NC):
        t0 = c * CHUNK
        qidx, _ = _q_slices(c)
        in_maps.append({
            **shared,
            "xT_kv": np.ascontiguousarray(xT_full[:, t0:t0 + CHUNK]).astype(bf16),
            "xT_q": np.ascontiguousarray(xT_full[:, qidx]).astype(bf16),
            "costk": np.ascontiguousarray(costab[:, t0:t0 + CHUNK]),
            "sintk": np.ascontiguousarray(sintab[:, t0:t0 + CHUNK]),
            "costq": np.ascontiguousarray(costab[:, qidx]),
            "sintq": np.ascontiguousarray(sintab[:, qidx]),
        })
    return key, in_maps


def _assemble(results):
    out = np.empty((1, T, D), np.float32)
    for c in range(NC):
        qidx, cnt = _q_slices(c)
        part = results[c]["out_part"]
        for f in range(4):
            toks = qidx[f * QG:f * QG + cnt]
            out[0, toks, :] = part[f * QG:f * QG + cnt]
    return out


def kernel(x, freqs_cos, freqs_sin, Wq, bq, Wk, bk, Wv, bv, Wo, bo,
           gq, gk, frame_seqlen):
    key, in_maps = _prep_inputs(x, freqs_cos, freqs_sin, Wq, bq, Wk, bk,
                                Wv, bv, Wo, bo, gq, gk, frame_seqlen)
    nc = _get_nc(key)
    res = run_bass_kernel_spmd(nc, in_maps, core_ids=list(range(NC)))
    return _assemble(res.results)


# revision 19
# speedup vs baseline: 1.5167x; 1.0651x over previous
"""Trainium2 Bass kernel for nn_CausalWanModel (frame-block-causal attention).

Self-contained: hardcodes shapes from the problem spec.
  B=1, T=3120, D=1536, H=12 heads, hd=128, frame_seqlen=780, 8 cores.

Sharding:
  k/v projections: sequence-parallel, core c owns tokens [390c, 390c+390).
    k/v are exchanged via AllGathers split in thirds (head groups of 4 for
    k, column groups of 512 for v) so the collective pipeline overlaps the
    q projection and early attention heads.
  queries: interleaved frame sharding. Each frame of 780 tokens is split
    across the 8 cores ([780f + 390c/4, 780f + 390(c+1)/4)), so every core
    owns 4 "frame groups" of <=98 query tokens, one per frame. The
    frame-block-causal mask then becomes a static key-prefix length per
    group (frame f attends to keys [0, 780(f+1))) -- no mask tensors, no
    wasted matmul/exp/add work, and the program is identical on all cores.

Keys are tiled 24x128+48 over the gathered 3120 tokens (no padding). The
only partial tiles are the frame-boundary tiles {6,12,18,24} where just
the first {12,24,36,48} key rows participate for the ending frame.

Matmuls run in bf16 (fp32 PSUM accumulation); RMSNorm statistics in fp32.
"""

import math

import numpy as np
import ml_dtypes

import concourse.bacc as bacc
import concourse.mybir as mybir
import concourse.tile as tile
from concourse.bass_utils import run_bass_kernel_spmd

F32 = mybir.dt.float32
BF16 = mybir.dt.bfloat16

NC = 8
T = 3120
D = 1536
H = 12
HD = 128
L = 780  # frame_seqlen
CHUNK = T // NC  # 390 kv tokens per core
KC = D // 128  # 12 contraction chunks
EPS = 1e-6
SCALE = 1.0 / math.sqrt(HD)

QG = 98                 # query slots per frame group (97 or 98 real)
QW = 4 * QG             # 392 query slots per core
NKT = 25                # key tiles over 3120 gathered keys: 24x128 + 48
KSZ = [128] * 24 + [48]
BT = [6, 12, 18, 24]    # frame f's boundary (partial) key tile
NV = [12, 24, 36, 48]   # valid rows of that partial tile
COLG = [(g * 512, 512) for g in range(3)]
TOK_SUBS_KV = [(0, 128), (128, 128), (256, 128), (384, 6)]
TOK_SUBS_Q = [(0, 128), (128, 128), (256, 128), (384, 8)]


def _fmin(kt):
    """Smallest frame group treating key tile kt as full, else None."""
    for f in range(4):
        if kt < BT[f]:
            return f
    return None


def _pfrm(kt):
    """Frame group for which kt is the partial boundary tile, else None."""
    return BT.index(kt) if kt in BT else None


def build_kernel(apply_bias_qk=False, apply_g=False, apply_bias_v=False,
                 apply_bias_o=False, debug=False):
    nc = bacc.Bacc("TRN2", target_bir_lowering=False, debug=False, num_devices=NC)

    # ---- I/O ----
    xT_kv = nc.dram_tensor("xT_kv", [D, CHUNK], BF16, kind="ExternalInput")
    xT_q = nc.dram_tensor("xT_q", [D, QW], BF16, kind="ExternalInput")
    # weights pre-tiled on host: wq/wk[d] = [128, KC*128] (lhsT chunks),
    # wv/wo[g] = [128, KC*512] (rhs chunks per column group)
    wq = nc.dram_tensor("wq", [KC, 128, KC * 128], BF16, kind="ExternalInput")
    wk = nc.dram_tensor("wk", [KC, 128, KC * 128], BF16, kind="ExternalInput")
    wv = nc.dram_tensor("wv", [3, 128, KC * 512], BF16, kind="ExternalInput")
    wo = nc.dram_tensor("wo", [3, 128, KC * 512], BF16, kind="ExternalInput")
    costk = nc.dram_tensor("costk", [128, CHUNK], F32, kind="ExternalInput")
    sintk = nc.dram_tensor("sintk", [128, CHUNK], F32, kind="ExternalInput")
    costq = nc.dram_tensor("costq", [128, QW], F32, kind="ExternalInput")
    sintq = nc.dram_tensor("sintq", [128, QW], F32, kind="ExternalInput")
    bqk2 = nc.dram_tensor("bqk2", [2 * KC, 128], F32, kind="ExternalInput")
    gqk2 = nc.dram_tensor("gqk2", [2 * KC, 128], F32, kind="ExternalInput")
    bvo = nc.dram_tensor("bvo", [2, D], F32, kind="ExternalInput")
    out_part = nc.dram_tensor("out_part", [QW, D], F32, kind="ExternalOutput")

    # ---- collective buffers (k in thirds by head group, v by col group) ----
    k_in = [nc.dram_tensor(f"k_in{i}", [512 * CHUNK], BF16) for i in range(3)]
    v_in = [nc.dram_tensor(f"v_in{i}", [CHUNK * 512], BF16) for i in range(3)]
    k_out = [nc.dram_tensor(f"k_out{i}", [NC, 512 * CHUNK], BF16,
                            addr_space="Shared") for i in range(3)]
    v_out = [nc.dram_tensor(f"v_out{i}", [NC, CHUNK * 512], BF16,
                            addr_space="Shared") for i in range(3)]

    if debug:
        dbg_qT = nc.dram_tensor("dbg_qT", [128, KC * QW], F32, kind="ExternalOutput")
        dbg_kT = nc.dram_tensor("dbg_kT", [128, KC * CHUNK], F32,
                                kind="ExternalOutput")
        dbg_sums = nc.dram_tensor("dbg_sums", [H, QW], F32, kind="ExternalOutput")

    with tile.TileContext(nc) as tc:
        with tc.tile_pool(name="const", bufs=1) as cpool:
            xkv_sb = cpool.tile([128, KC * CHUNK], BF16, tag="xkv_sb")
            xq_sb = cpool.tile([128, KC * QW], BF16, tag="xq_sb")
            qT_sb = cpool.tile([128, H * QW], BF16, tag="qT_sb")
            attnT_sb = cpool.tile([128, H * QW], BF16, tag="attnT_sb")
            costk_sb = cpool.tile([128, CHUNK], F32, tag="costk_sb")
            sintk_sb = cpool.tile([128, CHUNK], F32, tag="sintk_sb")
            costq_sb = cpool.tile([128, QW], F32, tag="costq_sb")
            sintq_sb = cpool.tile([128, QW], F32, tag="sintq_sb")
            ones_f32 = cpool.tile([128, 1], F32, tag="ones_f32")
            ones_bf = cpool.tile([128, 1], BF16, tag="ones_bf")
            ones_row = cpool.tile([1, 128], F32, tag="ones_row")
            nc.gpsimd.memset(ones_row[:, :], 1.0)
            sq_scale = cpool.tile([1, QW], F32, tag="sq_scale")
            sk_scale = cpool.tile([1, CHUNK], F32, tag="sk_scale")
            sq_bc = cpool.tile([128, QW], F32, tag="sq_bc")
            sk_bc = cpool.tile([128, CHUNK], F32, tag="sk_bc")

            eps_sb = cpool.tile([1, 1], F32, tag="eps_sb")
            nc.gpsimd.memset(ones_f32[:, :], 1.0)
            nc.gpsimd.memset(ones_bf[:, :], 1.0)
            nc.gpsimd.memset(eps_sb[:, :], EPS)

            for d in range(KC):
                nc.scalar.dma_start(out=xkv_sb[:, d * CHUNK:(d + 1) * CHUNK],
                                    in_=xT_kv[d * 128:(d + 1) * 128, :])
                nc.scalar.dma_start(out=xq_sb[:, d * QW:(d + 1) * QW],
                                    in_=xT_q[d * 128:(d + 1) * 128, :])
            nc.sync.dma_start(out=costk_sb[:, :], in_=costk[:, :])
            nc.sync.dma_start(out=sintk_sb[:, :], in_=sintk[:, :])
            nc.sync.dma_start(out=costq_sb[:, :], in_=costq[:, :])
            nc.sync.dma_start(out=sintq_sb[:, :], in_=sintq[:, :])
            bqk_sb = gqk_sb = bvo_sb = None
            if apply_bias_qk:
                bqk_sb = cpool.tile([128, 2 * KC], F32, tag="bqk_sb")
                nc.sync.dma_start(out=bqk_sb[:, :],
                                  in_=bqk2.ap().rearrange("c p -> p c"))
            if apply_g:
                gqk_sb = cpool.tile([128, 2 * KC], F32, tag="gqk_sb")
                nc.sync.dma_start(out=gqk_sb[:, :],
                                  in_=gqk2.ap().rearrange("c p -> p c"))
            if apply_bias_v or apply_bias_o:
                bvo_sb = cpool.tile([2, D], F32, tag="bvo_sb")
                nc.sync.dma_start(out=bvo_sb[:, :], in_=bvo[:, :])

            # ===== Phase 1: projections + rmsnorm + rope =====
            with tc.tile_pool(name="p1sb", bufs=3) as p1sb, \
                 tc.tile_pool(name="p1w", bufs=5) as p1w, \
                 tc.tile_pool(name="upool", bufs=1) as upool, \
                 tc.tile_pool(name="p1ps", bufs=3, space="PSUM") as p1ps, \
                 tc.tile_pool(name="bcps", bufs=1, space="PSUM") as bcps, \
                 tc.tile_pool(name="ssqps", bufs=1, space="PSUM") as ssqps:

                u_tiles = {(name, d): upool.tile(
                    [128, QW if name == "q" else CHUNK], F32,
                    name=f"u_{name}_{d}", tag=f"u_{name}_{d}")
                           for name in ("q", "k") for d in range(KC)}
                ssq_ps = {}

                def qk_proj(name, w, is_q):
                    width = QW if is_q else CHUNK
                    xsrc = xq_sb if is_q else xkv_sb
                    ssq_ps[name] = ssqps.tile([1, width], F32, name=f"ssq_{name}",
                                              tag="ssq")
                    for d in range(KC):
                        wt = p1w.tile([128, D], BF16, tag="wqk_t", name="wqk_t")
                        nc.sync.dma_start(out=wt[:, :], in_=w[d, :, :])
                        ps = p1ps.tile([128, width], F32, tag="proj_ps",
                                       name="proj_ps")
                        for c in range(KC):
                            nc.tensor.matmul(
                                ps[:, :],
                                wt[:, c * 128:(c + 1) * 128],
                                xsrc[:, c * width:(c + 1) * width],
                                start=(c == 0), stop=(c == KC - 1))
                        ur = u_tiles[(name, d)]
                        if apply_bias_qk:
                            bias_col = (0 if is_q else KC) + d
                            nc.vector.tensor_scalar_add(
                                ur[:, :], ps[:, :], bqk_sb[:, bias_col:bias_col + 1])
                        else:
                            nc.scalar.copy(ur[:, :], ps[:, :])
                        sq = p1sb.tile([128, width], BF16, tag="sqsb", name="sqsb")
                        nc.vector.tensor_tensor(sq[:, :], ur[:, :], ur[:, :],
                                                mybir.AluOpType.mult)
                        nc.tensor.matmul(ssq_ps[name][:, :], ones_bf[:, :], sq[:, :],
                                         start=(d == 0), stop=(d == KC - 1))

                def qk_scales(name, stile, sbc):
                    width = QW if name == "q" else CHUNK
                    nc.scalar.activation(stile[:, :], ssq_ps[name][:, :],
                                         mybir.ActivationFunctionType.Sqrt,
                                         bias=eps_sb[:, :], scale=1.0 / D)
                    nc.vector.reciprocal_approx_fast(stile[:, :], stile[:, :])
                    # broadcast across partitions via PE (gpsimd is busy
                    # triggering collectives; don't queue behind them)
                    bc_ps = bcps.tile([128, width], F32, name=f"bc_{name}",
                                      tag="bc")
                    nc.tensor.matmul(bc_ps[:, :], ones_row[:, :], stile[:, :],
                                     start=True, stop=True)
                    nc.scalar.copy(sbc[:, :], bc_ps[:, :])

                def qk_rope(name, sbc, d):
                    is_q = name == "q"
                    width = QW if is_q else CHUNK
                    cost_sb = costq_sb if is_q else costk_sb
                    sint_sb = sintq_sb if is_q else sintk_sb
                    ur = u_tiles[(name, d)]
                    qs = p1sb.tile([128, width], F32, tag="qs", name="qs")
                    nc.vector.tensor_tensor(qs[:, :], ur[:, :], sbc[:, :],
                                            mybir.AluOpType.mult)
                    if apply_g:
                        gcol = (0 if is_q else KC) + d
                        nc.vector.tensor_scalar_mul(
                            qs[:, :], qs[:, :], gqk_sb[:, gcol:gcol + 1])
                    qsw = p1sb.tile([128, width], F32, tag="qsw", name="qsw")
                    nc.scalar.dma_start(out=qsw[0:64, :], in_=qs[64:128, :])
                    nc.scalar.dma_start(out=qsw[64:128, :], in_=qs[0:64, :])
                    t1 = p1sb.tile([128, width], F32, tag="rope_t1", name="rope_t1")
                    t2 = p1sb.tile([128, width], F32, tag="rope_t2", name="rope_t2")
                    nc.vector.tensor_tensor(t1[:, :], qs[:, :], cost_sb[:, :],
                                            mybir.AluOpType.mult)
                    nc.vector.tensor_tensor(t2[:, :], qsw[:, :], sint_sb[:, :],
                                            mybir.AluOpType.mult)
                    if is_q:
                        dst = qT_sb[:, d * QW:(d + 1) * QW]
                        nc.vector.tensor_tensor(dst[:, :], t1[:, :], t2[:, :],
                                                mybir.AluOpType.add)
                        if debug:
                            df = p1sb.tile([128, width], F32, tag="dbgf",
                                           name="dbgf")
                            nc.vector.tensor_copy(df[:, :], dst)
                            nc.sync.dma_start(
                                out=dbg_qT[:, d * QW:(d + 1) * QW], in_=df[:, :])
                    else:
                        kr = p1sb.tile([128, width], BF16, tag="krope",
                                       name="krope")
                        nc.vector.tensor_tensor(kr[:, :], t1[:, :], t2[:, :],
                                                mybir.AluOpType.add)
                        # k_in third i holds k rows [512i, 512i+512) x CHUNK
                        third = d // 4
                        row0 = (d % 4) * 128
                        nc.sync.dma_start(
                            out=k_in[third].ap()
                            .rearrange("(r t) -> r t", t=CHUNK)
                            [row0:row0 + 128, :],
                            in_=kr[:, :])
                        if debug:
                            df = p1sb.tile([128, width], F32, tag="dbgf",
                                           name="dbgf")
                            nc.vector.tensor_copy(df[:, :], kr[:, :])
                            nc.sync.dma_start(
                                out=dbg_kT[:, d * CHUNK:(d + 1) * CHUNK],
                                in_=df[:, :])

                def allgather(src, dst):
                    nc.gpsimd.collective_compute(
                        "AllGather", mybir.AluOpType.bypass,
                        ins=[src.ap().opt()],
                        outs=[dst.ap().opt()],
                        replica_groups=[list(range(NC))],
                    )

                # ---- k first: proj, scales, rope in thirds, AG per third ----
                qk_proj("k", wk, False)
                qk_scales("k", sk_scale, sk_bc)
                for d in range(4):
                    qk_rope("k", sk_bc, d)
                allgather(k_in[0], k_out[0])
                for d in range(4, 8):
                    qk_rope("k", sk_bc, d)

                # ---- v projection per column group, AG after each ----
                for gi, (c0, csz) in enumerate(COLG):
                    wt = p1w.tile([128, KC * 512], BF16, tag="wv_t", name="wv_t")
                    nc.sync.dma_start(out=wt[:, :], in_=wv[gi, :, :])
                    for (t0, tsz) in TOK_SUBS_KV:
                        ps = p1ps.tile([128, 512], F32, tag="v_ps", name="v_ps")
                        for c in range(KC):
                            nc.tensor.matmul(
                                ps[0:tsz, :],
                                xkv_sb[:, c * CHUNK + t0:c * CHUNK + t0 + tsz],
                                wt[:, c * 512:(c + 1) * 512],
                                start=(c == 0), stop=(c == KC - 1))
                        vsb = p1sb.tile([128, 512], BF16, tag="vsb", name="vsb")
                        if apply_bias_v:
                            bvb = p1sb.tile([128, 512], F32, tag="bvb", name="bvb")
                            nc.gpsimd.partition_broadcast(
                                bvb[:, :], bvo_sb[0:1, c0:c0 + csz])
                            nc.vector.tensor_tensor(
                                vsb[0:tsz, :], ps[0:tsz, :], bvb[0:tsz, :],
                                mybir.AluOpType.add)
                        else:
                            nc.vector.tensor_copy(vsb[0:tsz, :], ps[0:tsz, :])
                        nc.sync.dma_start(
                            out=v_in[gi].ap().rearrange("(t c) -> t c", c=512)
                            [t0:t0 + tsz, :],
                            in_=vsb[0:tsz, :])
                    if gi == 0:
                        # finish remaining k rope, then stream AGs:
                        # k0 (already queued), v0, k1, v1, k2, v2
                        for d in range(8, KC):
                            qk_rope("k", sk_bc, d)
                        allgather(v_in[0], v_out[0])
                        allgather(k_in[1], k_out[1])
                    elif gi == 1:
                        allgather(v_in[1], v_out[1])
                        allgather(k_in[2], k_out[2])
                    else:
                        allgather(v_in[2], v_out[2])

                # ---- q last (overlaps the collectives) ----
                qk_proj("q", wq, True)
                qk_scales("q", sq_scale, sq_bc)
                for d in range(KC):
                    qk_rope("q", sq_bc, d)

            # =========== Phase 2: attention ===========
            with tc.tile_pool(name="p4w", bufs=1) as p4w:
              with tc.tile_pool(name="a_k", bufs=2) as akp, \
                   tc.tile_pool(name="a_v", bufs=2) as avp, \
                   tc.tile_pool(name="a_p", bufs=4) as app, \
                   tc.tile_pool(name="a_sb", bufs=3) as asb, \
                   tc.tile_pool(name="a_ps", bufs=2, space="PSUM") as aps, \
                   tc.tile_pool(name="acc_ps", bufs=2, space="PSUM") as accps, \
                   tc.tile_pool(name="bca_ps", bufs=1, space="PSUM") as bcaps, \
                   tc.tile_pool(name="sum_ps", bufs=1, space="PSUM") as sumps:
                # prefetch Wo column groups during attention
                wo_tiles = []
                for gi, (c0, csz) in enumerate(COLG):
                    wt = p4w.tile([128, KC * 512], BF16, tag=f"wo_t{gi}",
                                  name=f"wo_t{gi}")
                    nc.sync.dma_start(out=wt[:, :], in_=wo[gi, :, :])
                    wo_tiles.append(wt)

                for h in range(H):
                    # keys for head h: [128 hd, 3120 keys]
                    kthird = h // 4
                    krow = (h % 4) * 128
                    kt_sb = akp.tile([128, T], BF16, tag="kt_sb", name="kt_sb")
                    for r in range(NC):
                        nc.sync.dma_start(
                            out=kt_sb[:, r * CHUNK:(r + 1) * CHUNK],
                            in_=k_out[kthird].ap()[r, :]
                            .rearrange("(row t) -> row t", t=CHUNK)
                            [krow:krow + 128, :])
                    # values for head h: [128 slot-in-tile, 25 tiles, 128 hd]
                    vthird = (h * 128) // 512
                    vcol = (h * 128) % 512
                    vt_sb = avp.tile([128, NKT, 128], BF16, tag="vt_sb",
                                     name="vt_sb")
                    vflat = (v_out[vthird].ap()
                             .rearrange("r (t c) -> (r t) c", c=512)
                             [:, vcol:vcol + 128])
                    nc.sync.dma_start(
                        out=vt_sb[:, 0:24, :],
                        in_=vflat[0:3072, :].rearrange("(t p) c -> p t c", p=128))
                    nc.sync.dma_start(
                        out=vt_sb[0:48, 24, :], in_=vflat[3072:T, :])

                    qh = qT_sb[:, h * QW:(h + 1) * QW]
                    acc = accps.tile([128, QW], F32, tag="acc", name="acc")
                    sumacc = asb.tile([128, QW], F32, tag="sumacc", name="sumacc")
                    nc.vector.memset(sumacc[:, :], 0.0)

                    for b in range(13):  # key-tile batches {0,1},...,{24}
                        kts = [2 * b, 2 * b + 1] if b < 12 else [24]
                        sc = aps.tile([128, 2, 512], F32, tag="sc", name="sc")
                        fmin = _fmin(kts[0])  # same within a batch (or None)
                        for j, kt in enumerate(kts):
                            ksz = KSZ[kt]
                            if fmin is not None:
                                nc.tensor.matmul(
                                    sc[0:ksz, j, QG * fmin:QW],
                                    kt_sb[:, 128 * kt:128 * kt + ksz],
                                    qh[:, QG * fmin:QW],
                                    start=True, stop=True)
                            pf = _pfrm(kt)
                            if pf is not None:
                                nvv = NV[pf]
                                nc.tensor.matmul(
                                    sc[0:nvv, j, QG * pf:QG * pf + QG],
                                    kt_sb[:, 128 * kt:128 * kt + nvv],
                                    qh[:, QG * pf:QG * pf + QG],
                                    start=True, stop=True)
                        pr = app.tile([128, 2, QW], BF16, tag="pr", name="pr")
                        if fmin is not None:
                            nc.scalar.activation(
                                pr[:, 0:len(kts), QG * fmin:QW],
                                sc[:, 0:len(kts), QG * fmin:QW],
                                mybir.ActivationFunctionType.Exp, scale=SCALE)
                        for j, kt in enumerate(kts):
                            pf = _pfrm(kt)
                            if pf is not None:
                                nvv = NV[pf]
                                nc.scalar.activation(
                                    pr[0:nvv, j, QG * pf:QG * pf + QG],
                                    sc[0:nvv, j, QG * pf:QG * pf + QG],
                                    mybir.ActivationFunctionType.Exp, scale=SCALE)
                        for j, kt in enumerate(kts):
                            ksz = KSZ[kt]
                            if fmin is not None:
                                nc.tensor.matmul(
                                    acc[:, QG * fmin:QW],
                                    vt_sb[0:ksz, kt, :],
                                    pr[0:ksz, j, QG * fmin:QW],
                                    start=(kt == 0), stop=False)
                                nc.vector.tensor_tensor(
                                    sumacc[0:ksz, QG * fmin:QW],
                                    sumacc[0:ksz, QG * fmin:QW],
                                    pr[0:ksz, j, QG * fmin:QW],
                                    mybir.AluOpType.add)
                            pf = _pfrm(kt)
                            if pf is not None:
                                nvv = NV[pf]
                                nc.tensor.matmul(
                                    acc[:, QG * pf:QG * pf + QG],
                                    vt_sb[0:nvv, kt, :],
                                    pr[0:nvv, j, QG * pf:QG * pf + QG],
                                    start=False, stop=True)
                                nc.vector.tensor_tensor(
                                    sumacc[0:nvv, QG * pf:QG * pf + QG],
                                    sumacc[0:nvv, QG * pf:QG * pf + QG],
                                    pr[0:nvv, j, QG * pf:QG * pf + QG],
                                    mybir.AluOpType.add)

                    sums = sumps.tile([1, QW], F32, tag="sums", name="sums")
                    nc.tensor.matmul(sums[:, :], ones_f32[:, :], sumacc[:, :],
                                     start=True, stop=True)
                    rec = asb.tile([1, QW], F32, tag="rec", name="rec")
                    nc.scalar.copy(rec[:, :], sums[:, :])
                    nc.vector.reciprocal_approx_fast(rec[:, :], rec[:, :])
                    recb_ps = bcaps.tile([128, QW], F32, tag="recb_ps",
                                         name="recb_ps")
                    nc.tensor.matmul(recb_ps[:, :], ones_row[:, :], rec[:, :],
                                     start=True, stop=True)
                    recb = asb.tile([128, QW], F32, tag="recb", name="recb")
                    nc.scalar.copy(recb[:, :], recb_ps[:, :])
                    nc.vector.tensor_tensor(
                        attnT_sb[:, h * QW:(h + 1) * QW],
                        acc[:, :], recb[:, :],
                        mybir.AluOpType.mult)
                    if debug:
                        ssb = asb.tile([1, QW], F32, tag="ssb", name="ssb")
                        nc.vector.tensor_copy(ssb[:, :], sums[:, :])
                        nc.sync.dma_start(out=dbg_sums[h:h + 1, :], in_=ssb[:, :])

            # =========== Phase 3: o-projection ===========
              with tc.tile_pool(name="p4sb", bufs=3) as p4sb, \
                   tc.tile_pool(name="p4ps", bufs=3, space="PSUM") as p4ps:
                  for gi, (c0, csz) in enumerate(COLG):
                      wt = wo_tiles[gi]
                      for (t0, tsz) in TOK_SUBS_Q:
                          ps = p4ps.tile([128, 512], F32, tag="o_ps", name="o_ps")
                          for hh in range(KC):
                              nc.tensor.matmul(
                                  ps[0:tsz, :],
                                  attnT_sb[:, hh * QW + t0:hh * QW + t0 + tsz],
                                  wt[:, hh * 512:(hh + 1) * 512],
                                  start=(hh == 0), stop=(hh == KC - 1))
                          osb = p4sb.tile([128, 512], F32, tag="osb", name="osb")
                          if apply_bias_o:
                              bob = p4sb.tile([128, 512], F32, tag="bob",
                                              name="bob")
                              nc.gpsimd.partition_broadcast(
                                  bob[:, :], bvo_sb[1:2, c0:c0 + csz])
                              nc.vector.tensor_tensor(
                                  osb[0:tsz, :], ps[0:tsz, :], bob[0:tsz, :],
                                  mybir.AluOpType.add)
                          else:
                              nc.vector.tensor_copy(osb[0:tsz, :], ps[0:tsz, :])
                          nc.sync.dma_start(
                              out=out_part[t0:t0 + tsz, c0:c0 + csz],
                              in_=osb[0:tsz, :])

    nc.compile()
    return nc


_NC_CACHE = {}


def _get_nc(key):
    if key not in _NC_CACHE:
        _NC_CACHE[key] = build_kernel(*key)
    return _NC_CACHE[key]


def _q_slices(c):
    """Per-core query token indices: 4 frame groups of QG slots (junk-padded)."""
    lo, hi = (CHUNK * c) // 4, (CHUNK * (c + 1)) // 4  # within-frame range
    idx = []
    for f in range(4):
        toks = list(range(L * f + lo, L * f + hi))
        toks += [toks[-1]] * (QG - len(toks))  # pad junk slots
        idx.extend(toks)
    return np.array(idx), hi - lo


def _prep_inputs(x, freqs_cos, freqs_sin, Wq, bq, Wk, bk, Wv, bv, Wo, bo,
                 gq, gk, frame_seqlen, debug=False):
    assert int(frame_seqlen) == L
    x2d = np.asarray(x, np.float32).reshape(T, D)
    xT_full = np.ascontiguousarray(x2d.T)

    perm = np.concatenate([
        np.concatenate([np.arange(0, 128, 2), np.arange(1, 128, 2)]) + 128 * h
        for h in range(H)])
    Wqp = np.asarray(Wq, np.float32)[:, perm]
    Wkp = np.asarray(Wk, np.float32)[:, perm]
    bqp = np.asarray(bq, np.float32)[perm]
    bkp = np.asarray(bk, np.float32)[perm]
    gqp = np.asarray(gq, np.float32)[perm]
    gkp = np.asarray(gk, np.float32)[perm]

    cosT = np.asarray(freqs_cos, np.float32).T
    sinT = np.asarray(freqs_sin, np.float32).T
    costab = np.concatenate([cosT, cosT], 0)   # [128, T]
    sintab = np.concatenate([-sinT, sinT], 0)

    bf16 = ml_dtypes.bfloat16

    apply_bias_qk = not (np.all(bqp == 0) and np.all(bkp == 0))
    apply_g = not (np.all(gqp == 1) and np.all(gkp == 1))
    apply_bias_v = not np.all(np.asarray(bv) == 0)
    apply_bias_o = not np.all(np.asarray(bo) == 0)
    key = (apply_bias_qk, apply_g, apply_bias_v, apply_bias_o, debug)

    def tile_lhsT(w):  # [D, D] -> [KC, 128, KC*128]: out[d, p, c*128+m] = w[c*128+p, d*128+m]
        return np.ascontiguousarray(
            w.reshape(KC, 128, KC, 128).transpose(2, 1, 0, 3)
            .reshape(KC, 128, KC * 128))

    def tile_rhs(w):  # [D, D] -> [3, 128, KC*512]: out[g, p, c*512+m] = w[c*128+p, g*512+m]
        return np.ascontiguousarray(
            w.reshape(KC, 128, 3, 512).transpose(2, 1, 0, 3)
            .reshape(3, 128, KC * 512))

    shared = {
        "wq": tile_lhsT(Wqp).astype(bf16), "wk": tile_lhsT(Wkp).astype(bf16),
        "wv": tile_rhs(np.asarray(Wv, np.float32)).astype(bf16),
        "wo": tile_rhs(np.asarray(Wo, np.float32)).astype(bf16),
        "bqk2": np.concatenate([bqp, bkp]).reshape(2 * KC, 128),
        "gqk2": np.concatenate([gqp, gkp]).reshape(2 * KC, 128),
        "bvo": np.stack([np.asarray(bv, np.float32),
                         np.asarray(bo, np.float32)]),
    }
    in_maps = []
    for c in range(NC):
        t0 = c * CHUNK
        qidx, _ = _q_slices(c)
        in_maps.append({
            **shared,
            "xT_kv": np.ascontiguousarray(xT_full[:, t0:t0 + CHUNK]).astype(bf16),
            "xT_q": np.ascontiguousarray(xT_full[:, qidx]).astype(bf16),
            "costk": np.ascontiguousarray(costab[:, t0:t0 + CHUNK]),
            "sintk": np.ascontiguousarray(sintab[:, t0:t0 + CHUNK]),
            "costq": np.ascontiguousarray(costab[:, qidx]),
            "sintq": np.ascontiguousarray(sintab[:, qidx]),
        })
    return key, in_maps


def _assemble(results):
    out = np.empty((1, T, D), np.float32)
    for c in range(NC):
        qidx, cnt = _q_slices(c)
        part = results[c]["out_part"]
        for f in range(4):
            toks = qidx[f * QG:f * QG + cnt]
            out[0, toks, :] = part[f * QG:f * QG + cnt]
    return out


def kernel(x, freqs_cos, freqs_sin, Wq, bq, Wk, bk, Wv, bv, Wo, bo,
           gq, gk, frame_seqlen):
    key, in_maps = _prep_inputs(x, freqs_cos, freqs_sin, Wq, bq, Wk, bk,
                                Wv, bv, Wo, bo, gq, gk, frame_seqlen)
    nc = _get_nc(key)
    res = run_bass_kernel_spmd(nc, in_maps, core_ids=list(range(NC)))
    return _assemble(res.results)
